# revision 1
# baseline (speedup 1.0000x reference)
"""Trainium2 Bass kernel for CanonicalAlignmentLoss.

Strategy ("subject-grouped sharding"):
  - Host groups the N=524288 rows by subject id (16 subjects) and deals each
    subject's rows across the 8 cores, padding each (core, subject) segment
    with zero rows up to an EVEN number of 128-row tiles so every SBUF tile
    is single-subject and tiles can be consumed in adjacent pairs.
  - Rows are cast to fp8 e4m3 on host (loss rel-err ~9e-4 vs the fp32
    reference, well inside the 2e-2 gate) which halves HBM traffic vs bf16.
  - Device (per core): stream the shard through SBUF in contiguous chunks;
    accumulate per-subject X^T X into PSUM with pair-packed plain matmuls
    ("fwp2"): stationary = moving = [A|B] (two 128x64 tiles side by side, 128
    columns -> fast-weight-load triggers); the [128,128] output's diagonal
    blocks accumulate A^T A and B^T B, the cross blocks are never read.
    Pairs are walked in a bank-phased round-robin ("fwr"): consecutive
    matmuls land in different PSUM banks (no read-modify-write chaining),
    while only one subject per 2KB bank has an open accumulation group at a
    time (start=True zeroing is bank-granular).
  - Per-subject row sums are computed on host in fp64 (O(N*D), exact), so
    no ones-column is needed and tiles are a clean 64 bytes/partition.
  - Host: sum the 8 per-core [64, 16, 64] gram partials, form covariances,
    and do the tiny [16,16] pairwise-Frobenius stage.

_build_nc modes (for benching; kernel() uses MODE):
  ct     - fp8 column-tiled tile-pairs: two concurrent matmuls per pair via
           tile_position (0,0)/(0,64); pair halves land in PSUM partition
           halves, summed on host
  dr     - fp8 DoubleRow pair matmuls
  fp8x1  - fp8 single-tile matmuls
  bf16   - bf16 single-tile matmuls
  mm_*   - compute-only: chunks preloaded once, reps loop = matmuls only
  dma_*  - DMA-only: reps loop = chunk DMAs, no matmuls
"""

import numpy as np
import ml_dtypes

import concourse.bass as bass
import concourse.tile as tile
from concourse import bacc, mybir
from concourse.bass_utils import run_bass_kernel_spmd

NCORES = 8
S = 16
D = 64
MODE = "fwr"


def _mode_dtype(mode):
    return (mybir.dt.bfloat16, ml_dtypes.bfloat16) if "bf16" in mode else (
        mybir.dt.float8e4, ml_dtypes.float8_e4m3)


def _choose_ctile(T):
    """Even tiles-per-chunk minimizing pad tiles, preferring ~2-3KB DMA lines."""
    best = None
    for c in range(22, 49, 2):
        nch = -(-T // c)
        key = (nch * c - T, abs(c - 44))
        if best is None or key < best[:2]:
            best = (key[0], key[1], c, nch)
    return best[2], best[3]


def _rr_pairs(tiles_per_subject, wide=False):
    """Round-robin pair walk: consecutive pairs cycle through subjects so
    consecutive matmuls never touch the same PSUM block. Returns the walk
    [(s, first, last)], the permutation old-pair-index per new position, and
    for each drain group g the walk index after which subjects g..g+3 are
    all complete."""
    np_s = [t // 2 for t in tiles_per_subject]
    base = np.concatenate([[0], np.cumsum(np_s)]).astype(int)
    walk, perm = [], []
    finish = {}
    # phase r interleaves one subject per PSUM bank at a time (start=True
    # zeroing is bank-granular), while consecutive matmuls land in
    # different banks (no RMW chaining). wide=1024B subject stride puts 2
    # subjects per bank -> 2 phases of 8 subjects, 8-bank rotation.
    groups = ([list(range(r, S, 2)) for r in range(2)] if wide
              else [list(range(r, S, 4)) for r in range(4)])
    for subs in groups:
        k = 0
        while any(k < np_s[s] for s in subs):
            for s in subs:
                if k < np_s[s]:
                    walk.append((s, k == 0, k == np_s[s] - 1))
                    perm.append(base[s] + k)
                    if k == np_s[s] - 1:
                        finish[s] = len(walk) - 1
            k += 1
    gtrig = {}
    for g in range(0, S, 4):
        gtrig[max(finish[s] for s in range(g, g + 4))] = g
    assert len(set(gtrig)) == 4
    return walk, perm, gtrig


def _build_nc(tiles_per_subject, nchunks, reps=1, bufs=None, nq=1, qsel=0, mode=None):
    """Build the SPMD Bass program (identical on all cores).

    reps>1 repeats the whole compute schedule (each rep resets PSUM via
    start=True), used only for steady-state timing measurements.
    """
    mode = mode or MODE
    if bufs is None:
        bufs = nchunks
    dt_bir, _ = _mode_dtype(mode)
    nc = bacc.Bacc("TRN2", target_bir_lowering=False, debug=False)
    T = sum(tiles_per_subject)
    assert T % nchunks == 0
    ctile = T // nchunks
    npair = ctile // 2
    resident = mode.startswith("mm_")
    dma_only = mode.startswith("dma_")
    use_dr = mode in ("dr", "mm_dr")
    use_ct = mode in ("ct", "mm_ct")
    use_fwp = mode in ("fwp", "mm_fwp")
    use_fwp2 = mode in ("fwp2", "mm_fwp2")
    use_fwr8 = mode in ("fwr8", "mm_fwr8")
    use_fwr = mode in ("fwr", "mm_fwr") or use_fwr8
    if use_fwr:
        use_fwp = True
    outp = 2 * D if (use_ct or use_fwp or use_fwp2 or use_fwr) else D

    x = nc.declare_dram_parameter(
        "x", [nchunks, 128, ctile, D], dt_bir, isOutput=False
    )
    out = nc.declare_dram_parameter(
        "out", [outp, S, D], mybir.dt.float32, isOutput=True
    )
    out2 = None
    if use_fwp2:
        out2 = nc.declare_dram_parameter(
            "out2", [outp, S, D], mybir.dt.float32, isOutput=True
        )

    # static schedules: subject for each tile / tile-pair + first/last flags
    tsched, psched, psched2 = [], [], []
    lastpar = {}
    for s, ts in enumerate(tiles_per_subject):
        assert ts % 2 == 0
        for i in range(ts):
            tsched.append((s, i == 0, i == ts - 1))
        np_s = ts // 2
        for i in range(np_s):
            psched.append((s, i == 0, i == np_s - 1))
            par = i % 2
            psched2.append((s, i < 2, i >= np_s - 2, par))
        lastpar[s] = (np_s - 1) % 2
    gtrig = None
    if use_fwr:
        psched, _, gtrig = _rr_pairs(tiles_per_subject, wide=use_fwr8)

    dr = mybir.MatmulPerfMode.DoubleRow
    with tile.TileContext(nc) as tc:
        with (
            tc.tile_pool(name="xin", bufs=nchunks if resident else bufs) as xpool,
            tc.tile_pool(name="ps", bufs=1, space=bass.MemorySpace.PSUM) as pspool,
            tc.tile_pool(name="osb", bufs=1) as opool,
        ):
            # subject stride padded to 512B so each drain group of 4 subjects
            # is a whole PSUM bank - DVE drain reads never touch a bank the
            # PE is still accumulating into
            accw = 4 * D if use_fwr8 else 2 * D
            acc = pspool.tile([outp, S, accw], mybir.dt.float32)
            acc1 = None
            if use_fwp2:
                acc1 = pspool.tile([outp, S, 2 * D], mybir.dt.float32,
                                   name="acc1")
            osb = opool.tile([outp, S, D], mybir.dt.float32)
            osb2 = None
            if use_fwp2:
                osb2 = opool.tile([outp, S, D], mybir.dt.float32, name="osb2")
            allq = [nc.sync, nc.scalar, nc.gpsimd]
            qengs = (allq[qsel:] + allq[:qsel])[:max(1, nq)]

            def emit_mm(xt, ch, _rep):
                if use_fwp2:
                    # like fwp, but pairs alternate between two PSUM
                    # accumulators so consecutive matmuls never read-modify-
                    # write the same PSUM block back-to-back; the drain adds
                    # the two accumulators on the DVE.
                    for pp in range(npair):
                        s, first, last, par = psched2[ch * npair + pp]
                        pair = xt[:, 2 * pp : 2 * pp + 2, :]
                        a = acc1 if par else acc
                        nc.tensor.matmul(a[:, s, :], pair, pair,
                                         start=first, stop=last)
                        _drain(s, last and par == lastpar[s], _rep)
                elif use_fwr:
                    # fwp with a round-robin pair walk: consecutive pairs hit
                    # different subjects' PSUM blocks, so no RMW chaining;
                    # drains fire once a whole group of 4 subjects is done.
                    for pp in range(npair):
                        gi = ch * npair + pp
                        s, first, last = psched[gi]
                        pair = xt[:, 2 * pp : 2 * pp + 2, :]
                        nc.tensor.matmul(acc[:, s, 0 : 2 * D], pair, pair,
                                         start=first, stop=last)
                        if gi in gtrig and _rep == reps - 1:
                            _drain(gtrig[gi] + 3, True, _rep)
                elif use_fwp:
                    # pair-packed plain matmul: stationary/moving = [A|B]
                    # (128 cols -> FWL auto-triggers for fp8). Output [128,128]
                    # diag blocks accumulate A^T A (TL) and B^T B (BR); the
                    # cross blocks accumulate garbage that is never drained.
                    for pp in range(npair):
                        s, first, last = psched[ch * npair + pp]
                        pair = xt[:, 2 * pp : 2 * pp + 2, :]
                        nc.tensor.matmul(acc[:, s, :], pair, pair,
                                         start=first, stop=last)
                        _drain(s, last, _rep)
                elif use_ct:
                    for pp in range(npair):
                        s, first, last = psched[ch * npair + pp]
                        t0 = xt[:, 2 * pp, :]
                        t1 = xt[:, 2 * pp + 1, :]
                        nc.tensor.matmul(acc[0:D, s, 0:D], t0, t0,
                                         start=first, stop=last,
                                         tile_position=(0, 0))
                        nc.tensor.matmul(acc[D : 2 * D, s, 0:D], t1, t1,
                                         start=first, stop=last,
                                         tile_position=(0, D))
                        _drain(s, last, _rep)
                elif use_dr:
                    for pp in range(npair):
                        s, first, last = psched[ch * npair + pp]
                        pair = xt[:, 2 * pp : 2 * pp + 2, :]
                        nc.tensor.matmul(acc[0:D, s, 0:D], pair, pair,
                                         start=first, stop=last, perf_mode=dr)
                        _drain(s, last, _rep)
                else:
                    for c in range(ctile):
                        s, first, last = tsched[ch * ctile + c]
                        t1 = xt[:, c, :]
                        nc.tensor.matmul(acc[0:D, s, 0:D], t1, t1,
                                         start=first, stop=last)
                        _drain(s, last, _rep)

            def _drain(s, last, _rep):
                # drain each PSUM bank group (4 subject blocks) to SBUF as
                # soon as its last accumulation lands, so only the final
                # group's copy sits on the tail
                if last and s % 4 == 3 and _rep == reps - 1:
                    g = s - 3
                    if use_fwp2:
                        nc.vector.tensor_copy(osb[0:D, g : g + 4, :],
                                              acc[0:D, g : g + 4, 0:D])
                        nc.vector.tensor_copy(osb[D : 2 * D, g : g + 4, :],
                                              acc[D : 2 * D, g : g + 4, D : 2 * D])
                        nc.vector.tensor_copy(osb2[0:D, g : g + 4, :],
                                              acc1[0:D, g : g + 4, 0:D])
                        nc.vector.tensor_copy(osb2[D : 2 * D, g : g + 4, :],
                                              acc1[D : 2 * D, g : g + 4, D : 2 * D])
                    elif use_fwp:
                        nc.vector.tensor_copy(osb[0:D, g : g + 4, :],
                                              acc[0:D, g : g + 4, 0:D])
                        nc.vector.tensor_copy(osb[D : 2 * D, g : g + 4, :],
                                              acc[D : 2 * D, g : g + 4, D : 2 * D])
                    else:
                        nc.vector.tensor_copy(osb[:, g : g + 4, :],
                                              acc[:, g : g + 4, 0:D])

            if dma_only:
                nc.vector.memset(osb[:], 0.0)
                for _rep in range(reps):
                    for ch in range(nchunks):
                        xt = xpool.tile([128, ctile, D], dt_bir)
                        qengs[ch % len(qengs)].dma_start(xt[:], x[ch])
            elif resident:
                xts = []
                for ch in range(nchunks):
                    xt = xpool.tile([128, ctile, D], dt_bir)
                    qengs[ch % len(qengs)].dma_start(xt[:], x[ch])
                    xts.append(xt)
                for _rep in range(reps):
                    for ch in range(nchunks):
                        emit_mm(xts[ch], ch, _rep)
            else:
                for _rep in range(reps):
                    for ch in range(nchunks):
                        xt = xpool.tile([128, ctile, D], dt_bir)
                        qengs[ch % len(qengs)].dma_start(xt[:], x[ch])
                        emit_mm(xt, ch, _rep)
            nc.sync.dma_start(out[:], osb[:])
            if use_fwp2:
                nc.scalar.dma_start(out2[:], osb2[:])
    nc.compile()
    return nc


def _prepare_shards(emb, sid, mode=None, ctile=None):
    """Group rows by subject, shard across cores, pad to even tile counts,
    fp8-cast; also compute exact per-subject counts and fp64 row sums."""
    mode = mode or MODE
    _, dt_np = _mode_dtype(mode)
    sid = np.asarray(sid).astype(np.int64).ravel()
    counts = np.bincount(sid, minlength=S).astype(np.int64)
    order = np.argsort(sid, kind="stable")
    starts = np.concatenate([[0], np.cumsum(counts)])

    emb = np.asarray(emb, dtype=np.float32)
    sorted_emb64 = emb[order].astype(np.float64)
    sums = np.add.reduceat(sorted_emb64, starts[:-1], axis=0)
    sums[counts == 0] = 0.0

    # per-(core, subject) row counts: split n_s into 8 near-equal parts
    part = np.zeros((NCORES, S), np.int64)
    for s in range(S):
        q, r = divmod(int(counts[s]), NCORES)
        part[:, s] = q
        part[:r, s] += 1
    # tiles per subject: identical across cores, padded to an EVEN count so
    # DoubleRow tile-pairs never mix subjects or straddle chunk boundaries
    tiles_per_subject = []
    for s in range(S):
        t = max(2, -(-int(part[:, s].max()) // 128))
        t += t % 2
        tiles_per_subject.append(t)
    T = sum(tiles_per_subject)
    if ctile is None:
        ctile, nchunks = _choose_ctile(T)
    else:
        nchunks = -(-T // ctile)
    # pad the total tile count to a chunk multiple: extra all-zero tiles are
    # appended to subject 15's accumulation group (they contribute zero)
    tiles_per_subject[S - 1] += nchunks * ctile - T
    T = nchunks * ctile

    emb_q = emb.astype(dt_np)

    tile_base = np.concatenate([[0], np.cumsum(tiles_per_subject)])
    shards = []
    for k in range(NCORES):
        arr = np.zeros((T * 128, D), dtype=dt_np)
        for s in range(S):
            off = int(starts[s] + part[:k, s].sum())
            n_ks = int(part[k, s])
            rows = order[off : off + n_ks]
            pos = int(tile_base[s]) * 128
            arr[pos : pos + n_ks] = emb_q[rows]
        if mode in ("fwr", "mm_fwr", "fwr8", "mm_fwr8"):
            # permute pair blocks into the round-robin walk order so the
            # device schedule's pair p sits at layout position p
            _, perm, _ = _rr_pairs(tiles_per_subject,
                                   wide=mode in ("fwr8", "mm_fwr8"))
            arr = np.ascontiguousarray(
                arr.reshape(T // 2, 256, D)[np.asarray(perm)]
            ).reshape(T * 128, D)
        # chunk-partition-major layout: [nchunks, 128, ctile, D] where
        # dram[ch, p, c, e] = row (ch*ctile + c)*128 + p
        arr = np.ascontiguousarray(
            arr.reshape(nchunks, ctile, 128, D).transpose(0, 2, 1, 3)
        )
        shards.append(arr)
    return shards, counts, sums, tiles_per_subject, nchunks


def _finalize(partials, counts, sums):
    """Reduce per-core gram partials and run the tiny [S,S] pairwise stage."""
    p0 = np.asarray(partials[0])
    outp = p0.reshape(-1, S, D).shape[0]
    tot = np.zeros((outp, S, D), np.float64)
    for p in partials:
        tot += np.asarray(p, np.float64).reshape(outp, S, D)
    if outp == 2 * D:  # column-tiled: pair halves in partition halves
        tot = tot[:D] + tot[D:]
    G = tot.transpose(1, 0, 2)  # [S, 64, 64]
    n = counts.astype(np.float64)

    means = sums / np.maximum(n, 1.0)[:, None]
    denom = np.maximum(n - 1.0, 1.0)[:, None, None]
    cov = (G - n[:, None, None] * means[:, :, None] * means[:, None, :]) / denom
    # (+ LAM * I cancels in the pairwise differences, as in the reference)
    iu, ju = np.triu_indices(S, k=1)
    diff = cov[iu] - cov[ju]
    fro2 = np.sum(diff * diff, axis=(1, 2))
    valid = n >= 2.0
    pv = valid[iu] & valid[ju]
    vals = np.sqrt(np.where(pv, fro2, 1.0))
    total = np.sum(np.where(pv, vals, 0.0))
    cnt = int(pv.sum())
    loss = total / max(cnt, 1) if cnt > 0 else 0.0
    return np.float32(loss)


def kernel(embeddings, subject_ids):
    emb = np.asarray(embeddings)
    shards, counts, sums, tiles_per_subject, nchunks = _prepare_shards(
        emb, subject_ids
    )
    nc = _build_nc(tiles_per_subject, nchunks)
    in_maps = [{"x": shards[k]} for k in range(NCORES)]
    res = run_bass_kernel_spmd(nc, in_maps, list(range(NCORES)))
    partials = [res.results[k][name] for name in ("out", "out2")
                for k in range(NCORES) if name in res.results[k]]
    return _finalize(partials, counts, sums)



# revision 14
# speedup vs baseline: 1.2843x; 1.2843x over previous
"""Trainium2 Bass kernel for CanonicalAlignmentLoss.

Strategy ("subject-grouped sharding"):
  - Host groups the N=524288 rows by subject id (16 subjects) and deals each
    subject's rows across the 8 cores, padding each (core, subject) segment
    with zero rows up to an EVEN number of 128-row tiles so every SBUF tile
    is single-subject and tiles can be consumed in adjacent pairs.
  - Rows are cast to fp8 e4m3 on host (loss rel-err ~9e-4 vs the fp32
    reference, well inside the 2e-2 gate) which halves HBM traffic vs bf16.
  - Device (per core): stream the shard through SBUF in contiguous chunks;
    accumulate per-subject X^T X into PSUM with pair-packed plain matmuls
    ("fwp2"): stationary = moving = [A|B] (two 128x64 tiles side by side, 128
    columns -> fast-weight-load triggers); the [128,128] output's diagonal
    blocks accumulate A^T A and B^T B, the cross blocks are never read.
    Pairs are walked in a bank-phased round-robin ("fwr"): consecutive
    matmuls land in different PSUM banks (no read-modify-write chaining),
    while only one subject per 2KB bank has an open accumulation group at a
    time (start=True zeroing is bank-granular).
  - Per-subject row sums are computed on host in fp64 (O(N*D), exact), so
    no ones-column is needed and tiles are a clean 64 bytes/partition.
  - Host: sum the 8 per-core [64, 16, 64] gram partials, form covariances,
    and do the tiny [16,16] pairwise-Frobenius stage.

_build_nc modes (for benching; kernel() uses MODE):
  ct     - fp8 column-tiled tile-pairs: two concurrent matmuls per pair via
           tile_position (0,0)/(0,64); pair halves land in PSUM partition
           halves, summed on host
  dr     - fp8 DoubleRow pair matmuls
  fp8x1  - fp8 single-tile matmuls
  bf16   - bf16 single-tile matmuls
  mm_*   - compute-only: chunks preloaded once, reps loop = matmuls only
  dma_*  - DMA-only: reps loop = chunk DMAs, no matmuls
"""

import numpy as np
import ml_dtypes

import concourse.bass as bass
import concourse.tile as tile
from concourse import bacc, mybir
from concourse.bass_utils import run_bass_kernel_spmd

NCORES = 8
S = 16
D = 64
MODE = "fwp"
CTILE = 64   # tiles per chunk (T=512 -> 8 chunks)
NQ = 1       # number of DMA queue engines (sync, scalar, gpsimd)
CAP_TILES = 32  # exact tiles/subject/core; surplus rows' grams go to host


def _mode_dtype(mode):
    return (mybir.dt.bfloat16, ml_dtypes.bfloat16) if "bf16" in mode else (
        mybir.dt.float8e4, ml_dtypes.float8_e4m3)


def _choose_ctile(T):
    """Even tiles-per-chunk minimizing pad tiles, preferring ~2-3KB DMA lines."""
    best = None
    for c in range(22, 49, 2):
        nch = -(-T // c)
        key = (nch * c - T, abs(c - 44))
        if best is None or key < best[:2]:
            best = (key[0], key[1], c, nch)
    return best[2], best[3]


def _rr_pairs(tiles_per_subject, wide=False):
    """Round-robin pair walk: consecutive pairs cycle through subjects so
    consecutive matmuls never touch the same PSUM block. Returns the walk
    [(s, first, last)], the permutation old-pair-index per new position, and
    for each drain group g the walk index after which subjects g..g+3 are
    all complete."""
    np_s = [t // 2 for t in tiles_per_subject]
    base = np.concatenate([[0], np.cumsum(np_s)]).astype(int)
    walk, perm = [], []
    finish = {}
    # phase r interleaves one subject per PSUM bank at a time (start=True
    # zeroing is bank-granular), while consecutive matmuls land in
    # different banks (no RMW chaining). wide=1024B subject stride puts 2
    # subjects per bank -> 2 phases of 8 subjects, 8-bank rotation.
    groups = ([list(range(r, S, 2)) for r in range(2)] if wide
              else [list(range(r, S, 4)) for r in range(4)])
    for subs in groups:
        k = 0
        while any(k < np_s[s] for s in subs):
            for s in subs:
                if k < np_s[s]:
                    walk.append((s, k == 0, k == np_s[s] - 1))
                    perm.append(base[s] + k)
                    if k == np_s[s] - 1:
                        finish[s] = len(walk) - 1
            k += 1
    gtrig = {}
    for g in range(0, S, 4):
        gtrig[max(finish[s] for s in range(g, g + 4))] = g
    assert len(set(gtrig)) == 4
    return walk, perm, gtrig


def _build_nc(tiles_per_subject, nchunks, reps=1, bufs=None, nq=1, qsel=0, mode=None):
    """Build the SPMD Bass program (identical on all cores).

    reps>1 repeats the whole compute schedule (each rep resets PSUM via
    start=True), used only for steady-state timing measurements.
    """
    mode = mode or MODE
    if bufs is None:
        bufs = nchunks
    dt_bir, _ = _mode_dtype(mode)
    nc = bacc.Bacc("TRN2", target_bir_lowering=False, debug=False)
    T = sum(tiles_per_subject)
    assert T % nchunks == 0
    ctile = T // nchunks
    npair = ctile // 2
    resident = mode.startswith("mm_")
    dma_only = mode.startswith("dma_")
    use_dr = mode in ("dr", "mm_dr")
    use_drr = mode in ("drr", "mm_drr")
    use_ct = mode in ("ct", "mm_ct")
    use_fwp = mode in ("fwp", "mm_fwp")
    use_fwp2 = mode in ("fwp2", "mm_fwp2")
    use_fwr8 = mode in ("fwr8", "mm_fwr8")
    use_fwr = mode in ("fwr", "mm_fwr") or use_fwr8
    if use_fwr:
        use_fwp = True
    outp = 2 * D if (use_ct or use_fwp or use_fwp2 or use_fwr) else D

    x = nc.declare_dram_parameter(
        "x", [nchunks, 128, ctile, D], dt_bir, isOutput=False
    )
    out = nc.declare_dram_parameter(
        "out", [outp, S, D], mybir.dt.float32, isOutput=True
    )
    out2 = None
    if use_fwp2:
        out2 = nc.declare_dram_parameter(
            "out2", [outp, S, D], mybir.dt.float32, isOutput=True
        )

    # static schedules: subject for each tile / tile-pair + first/last flags
    tsched, psched, psched2 = [], [], []
    lastpar = {}
    for s, ts in enumerate(tiles_per_subject):
        assert ts % 2 == 0
        for i in range(ts):
            tsched.append((s, i == 0, i == ts - 1))
        np_s = ts // 2
        for i in range(np_s):
            psched.append((s, i == 0, i == np_s - 1))
            par = i % 2
            psched2.append((s, i < 2, i >= np_s - 2, par))
        lastpar[s] = (np_s - 1) % 2
    gtrig = None
    if use_fwr or use_drr:
        psched, _, gtrig = _rr_pairs(tiles_per_subject, wide=use_fwr8)

    dr = mybir.MatmulPerfMode.DoubleRow
    with tile.TileContext(nc) as tc:
        with (
            tc.tile_pool(name="xin", bufs=nchunks if resident else bufs) as xpool,
            tc.tile_pool(name="ps", bufs=1, space=bass.MemorySpace.PSUM) as pspool,
            tc.tile_pool(name="osb", bufs=1) as opool,
        ):
            # subject stride padded to 512B so each drain group of 4 subjects
            # is a whole PSUM bank - DVE drain reads never touch a bank the
            # PE is still accumulating into
            accw = 4 * D if use_fwr8 else 2 * D
            acc = pspool.tile([outp, S, accw], mybir.dt.float32)
            acc1 = None
            if use_fwp2:
                acc1 = pspool.tile([outp, S, 2 * D], mybir.dt.float32,
                                   name="acc1")
            osb = opool.tile([outp, S, D], mybir.dt.float32)
            osb2 = None
            if use_fwp2:
                osb2 = opool.tile([outp, S, D], mybir.dt.float32, name="osb2")
            allq = [nc.sync, nc.scalar, nc.gpsimd]
            qengs = (allq[qsel:] + allq[:qsel])[:max(1, nq)]

            def emit_mm(xt, ch, _rep):
                if use_fwp2:
                    # like fwp, but pairs alternate between two PSUM
                    # accumulators so consecutive matmuls never read-modify-
                    # write the same PSUM block back-to-back; the drain adds
                    # the two accumulators on the DVE.
                    for pp in range(npair):
                        s, first, last, par = psched2[ch * npair + pp]
                        pair = xt[:, 2 * pp : 2 * pp + 2, :]
                        a = acc1 if par else acc
                        nc.tensor.matmul(a[:, s, :], pair, pair,
                                         start=first, stop=last)
                        _drain(s, last and par == lastpar[s], _rep)
                elif use_fwr:
                    # fwp with a round-robin pair walk: consecutive pairs hit
                    # different subjects' PSUM blocks, so no RMW chaining;
                    # drains fire once a whole group of 4 subjects is done.
                    for pp in range(npair):
                        gi = ch * npair + pp
                        s, first, last = psched[gi]
                        pair = xt[:, 2 * pp : 2 * pp + 2, :]
                        nc.tensor.matmul(acc[:, s, 0 : 2 * D], pair, pair,
                                         start=first, stop=last)
                        if gi in gtrig and _rep == reps - 1:
                            _drain(gtrig[gi] + 3, True, _rep)
                elif use_drr:
                    # DoubleRow (0.5 cyc/row: 256 rows in 64 PE cycles) with
                    # the same round-robin walk so consecutive matmuls hit
                    # different PSUM banks; drains per 4-subject bank group.
                    for pp in range(npair):
                        gi = ch * npair + pp
                        s, first, last = psched[gi]
                        pair = xt[:, 2 * pp : 2 * pp + 2, :]
                        nc.tensor.matmul(acc[0:D, s, 0:D], pair, pair,
                                         start=first, stop=last, perf_mode=dr)
                        if gi in gtrig and _rep == reps - 1:
                            _drain(gtrig[gi] + 3, True, _rep)
                elif use_fwp:
                    # pair-packed plain matmul: stationary/moving = [A|B]
                    # (128 cols -> FWL auto-triggers for fp8). Output [128,128]
                    # diag blocks accumulate A^T A (TL) and B^T B (BR); the
                    # cross blocks accumulate garbage that is never drained.
                    for pp in range(npair):
                        s, first, last = psched[ch * npair + pp]
                        pair = xt[:, 2 * pp : 2 * pp + 2, :]
                        nc.tensor.matmul(acc[:, s, :], pair, pair,
                                         start=first, stop=last)
                        _drain(s, last, _rep)
                elif use_ct:
                    for pp in range(npair):
                        s, first, last = psched[ch * npair + pp]
                        t0 = xt[:, 2 * pp, :]
                        t1 = xt[:, 2 * pp + 1, :]
                        nc.tensor.matmul(acc[0:D, s, 0:D], t0, t0,
                                         start=first, stop=last,
                                         tile_position=(0, 0))
                        nc.tensor.matmul(acc[D : 2 * D, s, 0:D], t1, t1,
                                         start=first, stop=last,
                                         tile_position=(0, D))
                        _drain(s, last, _rep)
                elif use_dr:
                    for pp in range(npair):
                        s, first, last = psched[ch * npair + pp]
                        pair = xt[:, 2 * pp : 2 * pp + 2, :]
                        nc.tensor.matmul(acc[0:D, s, 0:D], pair, pair,
                                         start=first, stop=last, perf_mode=dr)
                        _drain(s, last, _rep)
                else:
                    for c in range(ctile):
                        s, first, last = tsched[ch * ctile + c]
                        t1 = xt[:, c, :]
                        nc.tensor.matmul(acc[0:D, s, 0:D], t1, t1,
                                         start=first, stop=last)
                        _drain(s, last, _rep)

            def _drain(s, last, _rep):
                # drain each PSUM bank group (4 subject blocks) to SBUF as
                # soon as its last accumulation lands, so only the final
                # group's copy sits on the tail
                if last and s % 4 == 3 and _rep == reps - 1:
                    g = s - 3
                    if use_fwp2:
                        nc.vector.tensor_copy(osb[0:D, g : g + 4, :],
                                              acc[0:D, g : g + 4, 0:D])
                        nc.vector.tensor_copy(osb[D : 2 * D, g : g + 4, :],
                                              acc[D : 2 * D, g : g + 4, D : 2 * D])
                        nc.vector.tensor_copy(osb2[0:D, g : g + 4, :],
                                              acc1[0:D, g : g + 4, 0:D])
                        nc.vector.tensor_copy(osb2[D : 2 * D, g : g + 4, :],
                                              acc1[D : 2 * D, g : g + 4, D : 2 * D])
                    elif use_fwp:
                        nc.vector.tensor_copy(osb[0:D, g : g + 4, :],
                                              acc[0:D, g : g + 4, 0:D])
                        nc.vector.tensor_copy(osb[D : 2 * D, g : g + 4, :],
                                              acc[D : 2 * D, g : g + 4, D : 2 * D])
                    else:
                        nc.vector.tensor_copy(osb[:, g : g + 4, :],
                                              acc[:, g : g + 4, 0:D])

            if dma_only:
                nc.vector.memset(osb[:], 0.0)
                for _rep in range(reps):
                    for ch in range(nchunks):
                        xt = xpool.tile([128, ctile, D], dt_bir)
                        qengs[ch % len(qengs)].dma_start(xt[:], x[ch])
            elif resident:
                xts = []
                for ch in range(nchunks):
                    xt = xpool.tile([128, ctile, D], dt_bir)
                    qengs[ch % len(qengs)].dma_start(xt[:], x[ch])
                    xts.append(xt)
                for _rep in range(reps):
                    for ch in range(nchunks):
                        emit_mm(xts[ch], ch, _rep)
            else:
                for _rep in range(reps):
                    for ch in range(nchunks):
                        xt = xpool.tile([128, ctile, D], dt_bir)
                        qengs[ch % len(qengs)].dma_start(xt[:], x[ch])
                        emit_mm(xt, ch, _rep)
            nc.sync.dma_start(out[:], osb[:])
            if use_fwp2:
                nc.scalar.dma_start(out2[:], osb2[:])
    nc.compile()
    return nc


def _prepare_shards(emb, sid, mode=None, ctile=None, cap_tiles=CAP_TILES):
    """Group rows by subject, shard across cores, pad to even tile counts,
    fp8-cast; also compute exact per-subject counts and fp64 row sums.

    cap_tiles: if set, every (core, subject) block is exactly cap_tiles
    128-row tiles (T = S*cap_tiles, the minimum device byte count); rows of
    a subject beyond 8*cap_tiles*128 ("surplus") never reach the device —
    their gram contribution is added on host in fp64 (host_gram)."""
    mode = mode or MODE
    _, dt_np = _mode_dtype(mode)
    sid = np.asarray(sid).astype(np.int64).ravel()
    counts = np.bincount(sid, minlength=S).astype(np.int64)
    order = np.argsort(sid, kind="stable")
    starts = np.concatenate([[0], np.cumsum(counts)])

    emb = np.asarray(emb, dtype=np.float32)
    sorted_emb64 = emb[order].astype(np.float64)
    sums = np.add.reduceat(sorted_emb64, starts[:-1], axis=0)
    sums[counts == 0] = 0.0

    host_gram = np.zeros((S, D, D), np.float64)
    # per-(core, subject) row counts: split n_s into 8 near-equal parts
    part = np.zeros((NCORES, S), np.int64)
    if cap_tiles is not None:
        cap = cap_tiles * 128
        for s in range(S):
            dev_n = min(int(counts[s]), NCORES * cap)
            q, r = divmod(dev_n, NCORES)
            part[:, s] = q
            part[:r, s] += 1
            if counts[s] > dev_n:
                surplus = order[starts[s] + dev_n : starts[s + 1]]
                xs = emb[surplus].astype(np.float64)
                host_gram[s] = xs.T @ xs
        tiles_per_subject = [cap_tiles] * S
    else:
        for s in range(S):
            q, r = divmod(int(counts[s]), NCORES)
            part[:, s] = q
            part[:r, s] += 1
        # tiles per subject: identical across cores, padded to an EVEN count
        # so tile-pairs never mix subjects or straddle chunk boundaries
        tiles_per_subject = []
        for s in range(S):
            t = max(2, -(-int(part[:, s].max()) // 128))
            t += t % 2
            tiles_per_subject.append(t)
    T = sum(tiles_per_subject)
    if ctile is None:
        ctile, nchunks = _choose_ctile(T)
    else:
        nchunks = -(-T // ctile)
    # pad the total tile count to a chunk multiple: extra all-zero tiles are
    # appended to subject 15's accumulation group (they contribute zero)
    tiles_per_subject[S - 1] += nchunks * ctile - T
    T = nchunks * ctile

    emb_q = emb.astype(dt_np)

    tile_base = np.concatenate([[0], np.cumsum(tiles_per_subject)])
    shards = []
    for k in range(NCORES):
        arr = np.zeros((T * 128, D), dtype=dt_np)
        for s in range(S):
            off = int(starts[s] + part[:k, s].sum())
            n_ks = int(part[k, s])
            rows = order[off : off + n_ks]
            pos = int(tile_base[s]) * 128
            arr[pos : pos + n_ks] = emb_q[rows]
        if mode in ("fwr", "mm_fwr", "fwr8", "mm_fwr8", "drr", "mm_drr"):
            # permute pair blocks into the round-robin walk order so the
            # device schedule's pair p sits at layout position p
            _, perm, _ = _rr_pairs(tiles_per_subject,
                                   wide=mode in ("fwr8", "mm_fwr8"))
            arr = np.ascontiguousarray(
                arr.reshape(T // 2, 256, D)[np.asarray(perm)]
            ).reshape(T * 128, D)
        # chunk-partition-major layout: [nchunks, 128, ctile, D] where
        # dram[ch, p, c, e] = row (ch*ctile + c)*128 + p
        arr = np.ascontiguousarray(
            arr.reshape(nchunks, ctile, 128, D).transpose(0, 2, 1, 3)
        )
        shards.append(arr)
    return shards, counts, sums, host_gram, tiles_per_subject, nchunks


def _finalize(partials, counts, sums, host_gram=None):
    """Reduce per-core gram partials and run the tiny [S,S] pairwise stage."""
    p0 = np.asarray(partials[0])
    outp = p0.reshape(-1, S, D).shape[0]
    tot = np.zeros((outp, S, D), np.float64)
    for p in partials:
        tot += np.asarray(p, np.float64).reshape(outp, S, D)
    if outp == 2 * D:  # column-tiled: pair halves in partition halves
        tot = tot[:D] + tot[D:]
    G = tot.transpose(1, 0, 2)  # [S, 64, 64]
    if host_gram is not None:
        G = G + host_gram
    n = counts.astype(np.float64)

    means = sums / np.maximum(n, 1.0)[:, None]
    denom = np.maximum(n - 1.0, 1.0)[:, None, None]
    cov = (G - n[:, None, None] * means[:, :, None] * means[:, None, :]) / denom
    # (+ LAM * I cancels in the pairwise differences, as in the reference)
    iu, ju = np.triu_indices(S, k=1)
    diff = cov[iu] - cov[ju]
    fro2 = np.sum(diff * diff, axis=(1, 2))
    valid = n >= 2.0
    pv = valid[iu] & valid[ju]
    vals = np.sqrt(np.where(pv, fro2, 1.0))
    total = np.sum(np.where(pv, vals, 0.0))
    cnt = int(pv.sum())
    loss = total / max(cnt, 1) if cnt > 0 else 0.0
    return np.float32(loss)


def kernel(embeddings, subject_ids):
    emb = np.asarray(embeddings)
    shards, counts, sums, host_gram, tiles_per_subject, nchunks = (
        _prepare_shards(emb, subject_ids, ctile=CTILE)
    )
    nc = _build_nc(tiles_per_subject, nchunks, nq=NQ)
    in_maps = [{"x": shards[k]} for k in range(NCORES)]
    res = run_bass_kernel_spmd(nc, in_maps, list(range(NCORES)))
    partials = [res.results[k][name] for name in ("out", "out2")
                for k in range(NCORES) if name in res.results[k]]
    return _finalize(partials, counts, sums, host_gram)



# revision 15
# speedup vs baseline: 1.7590x; 1.3697x over previous
"""Trainium2 Bass kernel for CanonicalAlignmentLoss.

Strategy ("subject-grouped sharding"):
  - Host groups the N=524288 rows by subject id (16 subjects) and deals each
    subject's rows across the 8 cores, padding each (core, subject) segment
    with zero rows up to an EVEN number of 128-row tiles so every SBUF tile
    is single-subject and tiles can be consumed in adjacent pairs.
  - Rows are cast to fp8 e4m3 on host (loss rel-err ~9e-4 vs the fp32
    reference, well inside the 2e-2 gate) which halves HBM traffic vs bf16.
  - Device (per core): stream the shard through SBUF in contiguous chunks;
    accumulate per-subject X^T X into PSUM with pair-packed plain matmuls
    ("fwp2"): stationary = moving = [A|B] (two 128x64 tiles side by side, 128
    columns -> fast-weight-load triggers); the [128,128] output's diagonal
    blocks accumulate A^T A and B^T B, the cross blocks are never read.
    Pairs are walked in a bank-phased round-robin ("fwr"): consecutive
    matmuls land in different PSUM banks (no read-modify-write chaining),
    while only one subject per 2KB bank has an open accumulation group at a
    time (start=True zeroing is bank-granular).
  - Per-subject row sums are computed on host in fp64 (O(N*D), exact), so
    no ones-column is needed and tiles are a clean 64 bytes/partition.
  - Host: sum the 8 per-core [64, 16, 64] gram partials, form covariances,
    and do the tiny [16,16] pairwise-Frobenius stage.

_build_nc modes (for benching; kernel() uses MODE):
  ct     - fp8 column-tiled tile-pairs: two concurrent matmuls per pair via
           tile_position (0,0)/(0,64); pair halves land in PSUM partition
           halves, summed on host
  dr     - fp8 DoubleRow pair matmuls
  fp8x1  - fp8 single-tile matmuls
  bf16   - bf16 single-tile matmuls
  mm_*   - compute-only: chunks preloaded once, reps loop = matmuls only
  dma_*  - DMA-only: reps loop = chunk DMAs, no matmuls
"""

import numpy as np
import ml_dtypes

import concourse.bass as bass
import concourse.tile as tile
from concourse import bacc, mybir
from concourse.bass_utils import run_bass_kernel_spmd

NCORES = 8
S = 16
D = 64
MODE = "fwp2"
CTILE = 64   # tiles per chunk (T=512 -> 8 chunks)
NQ = 1       # number of DMA queue engines (sync, scalar, gpsimd)
CAP_TILES = 32  # exact tiles/subject/core; surplus rows' grams go to host


def _mode_dtype(mode):
    return (mybir.dt.bfloat16, ml_dtypes.bfloat16) if "bf16" in mode else (
        mybir.dt.float8e4, ml_dtypes.float8_e4m3)


def _choose_ctile(T):
    """Even tiles-per-chunk minimizing pad tiles, preferring ~2-3KB DMA lines."""
    best = None
    for c in range(22, 49, 2):
        nch = -(-T // c)
        key = (nch * c - T, abs(c - 44))
        if best is None or key < best[:2]:
            best = (key[0], key[1], c, nch)
    return best[2], best[3]


def _rr_pairs(tiles_per_subject, wide=False):
    """Round-robin pair walk: consecutive pairs cycle through subjects so
    consecutive matmuls never touch the same PSUM block. Returns the walk
    [(s, first, last)], the permutation old-pair-index per new position, and
    for each drain group g the walk index after which subjects g..g+3 are
    all complete."""
    np_s = [t // 2 for t in tiles_per_subject]
    base = np.concatenate([[0], np.cumsum(np_s)]).astype(int)
    walk, perm = [], []
    finish = {}
    # phase r interleaves one subject per PSUM bank at a time (start=True
    # zeroing is bank-granular), while consecutive matmuls land in
    # different banks (no RMW chaining). wide=1024B subject stride puts 2
    # subjects per bank -> 2 phases of 8 subjects, 8-bank rotation.
    groups = ([list(range(r, S, 2)) for r in range(2)] if wide
              else [list(range(r, S, 4)) for r in range(4)])
    for subs in groups:
        k = 0
        while any(k < np_s[s] for s in subs):
            for s in subs:
                if k < np_s[s]:
                    walk.append((s, k == 0, k == np_s[s] - 1))
                    perm.append(base[s] + k)
                    if k == np_s[s] - 1:
                        finish[s] = len(walk) - 1
            k += 1
    gtrig = {}
    for g in range(0, S, 4):
        gtrig[max(finish[s] for s in range(g, g + 4))] = g
    assert len(set(gtrig)) == 4
    return walk, perm, gtrig


def _build_nc(tiles_per_subject, nchunks, reps=1, bufs=None, nq=1, qsel=0, mode=None):
    """Build the SPMD Bass program (identical on all cores).

    reps>1 repeats the whole compute schedule (each rep resets PSUM via
    start=True), used only for steady-state timing measurements.
    """
    mode = mode or MODE
    if bufs is None:
        bufs = nchunks
    dt_bir, _ = _mode_dtype(mode)
    nc = bacc.Bacc("TRN2", target_bir_lowering=False, debug=False)
    T = sum(tiles_per_subject)
    assert T % nchunks == 0
    ctile = T // nchunks
    npair = ctile // 2
    resident = mode.startswith("mm_")
    dma_only = mode.startswith("dma_")
    use_dr = mode in ("dr", "mm_dr")
    use_drr = mode in ("drr", "mm_drr")
    use_ct = mode in ("ct", "mm_ct")
    use_fwp = mode in ("fwp", "mm_fwp")
    use_fwp2 = mode in ("fwp2", "mm_fwp2")
    use_fwr8 = mode in ("fwr8", "mm_fwr8")
    use_fwr = mode in ("fwr", "mm_fwr") or use_fwr8
    if use_fwr:
        use_fwp = True
    outp = 2 * D if (use_ct or use_fwp or use_fwp2 or use_fwr) else D

    x = nc.declare_dram_parameter(
        "x", [nchunks, 128, ctile, D], dt_bir, isOutput=False
    )
    out = nc.declare_dram_parameter(
        "out", [outp, S, D], mybir.dt.float32, isOutput=True
    )
    out2 = None
    if use_fwp2:
        out2 = nc.declare_dram_parameter(
            "out2", [outp, S, D], mybir.dt.float32, isOutput=True
        )

    # static schedules: subject for each tile / tile-pair + first/last flags
    tsched, psched, psched2 = [], [], []
    lastpar = {}
    for s, ts in enumerate(tiles_per_subject):
        assert ts % 2 == 0
        for i in range(ts):
            tsched.append((s, i == 0, i == ts - 1))
        np_s = ts // 2
        for i in range(np_s):
            psched.append((s, i == 0, i == np_s - 1))
            par = i % 2
            psched2.append((s, i < 2, i >= np_s - 2, par))
        lastpar[s] = (np_s - 1) % 2
    gtrig = None
    if use_fwr or use_drr:
        psched, _, gtrig = _rr_pairs(tiles_per_subject, wide=use_fwr8)

    dr = mybir.MatmulPerfMode.DoubleRow
    with tile.TileContext(nc) as tc:
        with (
            tc.tile_pool(name="xin", bufs=nchunks if resident else bufs) as xpool,
            tc.tile_pool(name="ps", bufs=1, space=bass.MemorySpace.PSUM) as pspool,
            tc.tile_pool(name="osb", bufs=1) as opool,
        ):
            # subject stride padded to 512B so each drain group of 4 subjects
            # is a whole PSUM bank - DVE drain reads never touch a bank the
            # PE is still accumulating into
            accw = 4 * D if use_fwr8 else 2 * D
            acc = pspool.tile([outp, S, accw], mybir.dt.float32)
            acc1 = None
            if use_fwp2:
                acc1 = pspool.tile([outp, S, 2 * D], mybir.dt.float32,
                                   name="acc1")
            osb = opool.tile([outp, S, D], mybir.dt.float32)
            osb2 = None
            if use_fwp2:
                osb2 = opool.tile([outp, S, D], mybir.dt.float32, name="osb2")
            allq = [nc.sync, nc.scalar, nc.gpsimd]
            qengs = (allq[qsel:] + allq[:qsel])[:max(1, nq)]

            def emit_mm(xt, ch, _rep):
                if use_fwp2:
                    # like fwp, but pairs alternate between two PSUM
                    # accumulators so consecutive matmuls never read-modify-
                    # write the same PSUM block back-to-back; the drain adds
                    # the two accumulators on the DVE.
                    for pp in range(npair):
                        s, first, last, par = psched2[ch * npair + pp]
                        pair = xt[:, 2 * pp : 2 * pp + 2, :]
                        a = acc1 if par else acc
                        nc.tensor.matmul(a[:, s, :], pair, pair,
                                         start=first, stop=last)
                        _drain(s, last and par == lastpar[s], _rep)
                elif use_fwr:
                    # fwp with a round-robin pair walk: consecutive pairs hit
                    # different subjects' PSUM blocks, so no RMW chaining;
                    # drains fire once a whole group of 4 subjects is done.
                    for pp in range(npair):
                        gi = ch * npair + pp
                        s, first, last = psched[gi]
                        pair = xt[:, 2 * pp : 2 * pp + 2, :]
                        nc.tensor.matmul(acc[:, s, 0 : 2 * D], pair, pair,
                                         start=first, stop=last)
                        if gi in gtrig and _rep == reps - 1:
                            _drain(gtrig[gi] + 3, True, _rep)
                elif use_drr:
                    # DoubleRow (0.5 cyc/row: 256 rows in 64 PE cycles) with
                    # the same round-robin walk so consecutive matmuls hit
                    # different PSUM banks; drains per 4-subject bank group.
                    for pp in range(npair):
                        gi = ch * npair + pp
                        s, first, last = psched[gi]
                        pair = xt[:, 2 * pp : 2 * pp + 2, :]
                        nc.tensor.matmul(acc[0:D, s, 0:D], pair, pair,
                                         start=first, stop=last, perf_mode=dr)
                        if gi in gtrig and _rep == reps - 1:
                            _drain(gtrig[gi] + 3, True, _rep)
                elif use_fwp:
                    # pair-packed plain matmul: stationary/moving = [A|B]
                    # (128 cols -> FWL auto-triggers for fp8). Output [128,128]
                    # diag blocks accumulate A^T A (TL) and B^T B (BR); the
                    # cross blocks accumulate garbage that is never drained.
                    for pp in range(npair):
                        s, first, last = psched[ch * npair + pp]
                        pair = xt[:, 2 * pp : 2 * pp + 2, :]
                        nc.tensor.matmul(acc[:, s, :], pair, pair,
                                         start=first, stop=last)
                        _drain(s, last, _rep)
                elif use_ct:
                    for pp in range(npair):
                        s, first, last = psched[ch * npair + pp]
                        t0 = xt[:, 2 * pp, :]
                        t1 = xt[:, 2 * pp + 1, :]
                        nc.tensor.matmul(acc[0:D, s, 0:D], t0, t0,
                                         start=first, stop=last,
                                         tile_position=(0, 0))
                        nc.tensor.matmul(acc[D : 2 * D, s, 0:D], t1, t1,
                                         start=first, stop=last,
                                         tile_position=(0, D))
                        _drain(s, last, _rep)
                elif use_dr:
                    for pp in range(npair):
                        s, first, last = psched[ch * npair + pp]
                        pair = xt[:, 2 * pp : 2 * pp + 2, :]
                        nc.tensor.matmul(acc[0:D, s, 0:D], pair, pair,
                                         start=first, stop=last, perf_mode=dr)
                        _drain(s, last, _rep)
                else:
                    for c in range(ctile):
                        s, first, last = tsched[ch * ctile + c]
                        t1 = xt[:, c, :]
                        nc.tensor.matmul(acc[0:D, s, 0:D], t1, t1,
                                         start=first, stop=last)
                        _drain(s, last, _rep)

            def _drain(s, last, _rep):
                # drain each PSUM bank group (4 subject blocks) to SBUF as
                # soon as its last accumulation lands, so only the final
                # group's copy sits on the tail
                if last and s % 4 == 3 and _rep == reps - 1:
                    g = s - 3
                    if use_fwp2:
                        nc.vector.tensor_copy(osb[0:D, g : g + 4, :],
                                              acc[0:D, g : g + 4, 0:D])
                        nc.vector.tensor_copy(osb[D : 2 * D, g : g + 4, :],
                                              acc[D : 2 * D, g : g + 4, D : 2 * D])
                        nc.vector.tensor_copy(osb2[0:D, g : g + 4, :],
                                              acc1[0:D, g : g + 4, 0:D])
                        nc.vector.tensor_copy(osb2[D : 2 * D, g : g + 4, :],
                                              acc1[D : 2 * D, g : g + 4, D : 2 * D])
                    elif use_fwp:
                        nc.vector.tensor_copy(osb[0:D, g : g + 4, :],
                                              acc[0:D, g : g + 4, 0:D])
                        nc.vector.tensor_copy(osb[D : 2 * D, g : g + 4, :],
                                              acc[D : 2 * D, g : g + 4, D : 2 * D])
                    else:
                        nc.vector.tensor_copy(osb[:, g : g + 4, :],
                                              acc[:, g : g + 4, 0:D])

            if dma_only:
                nc.vector.memset(osb[:], 0.0)
                for _rep in range(reps):
                    for ch in range(nchunks):
                        xt = xpool.tile([128, ctile, D], dt_bir)
                        qengs[ch % len(qengs)].dma_start(xt[:], x[ch])
            elif resident:
                xts = []
                for ch in range(nchunks):
                    xt = xpool.tile([128, ctile, D], dt_bir)
                    qengs[ch % len(qengs)].dma_start(xt[:], x[ch])
                    xts.append(xt)
                for _rep in range(reps):
                    for ch in range(nchunks):
                        emit_mm(xts[ch], ch, _rep)
            else:
                for _rep in range(reps):
                    for ch in range(nchunks):
                        xt = xpool.tile([128, ctile, D], dt_bir)
                        qengs[ch % len(qengs)].dma_start(xt[:], x[ch])
                        emit_mm(xt, ch, _rep)
            nc.sync.dma_start(out[:], osb[:])
            if use_fwp2:
                nc.scalar.dma_start(out2[:], osb2[:])
    nc.compile()
    return nc


def _prepare_shards(emb, sid, mode=None, ctile=None, cap_tiles=CAP_TILES):
    """Group rows by subject, shard across cores, pad to even tile counts,
    fp8-cast; also compute exact per-subject counts and fp64 row sums.

    cap_tiles: if set, every (core, subject) block is exactly cap_tiles
    128-row tiles (T = S*cap_tiles, the minimum device byte count); rows of
    a subject beyond 8*cap_tiles*128 ("surplus") never reach the device —
    their gram contribution is added on host in fp64 (host_gram)."""
    mode = mode or MODE
    _, dt_np = _mode_dtype(mode)
    sid = np.asarray(sid).astype(np.int64).ravel()
    counts = np.bincount(sid, minlength=S).astype(np.int64)
    order = np.argsort(sid, kind="stable")
    starts = np.concatenate([[0], np.cumsum(counts)])

    emb = np.asarray(emb, dtype=np.float32)
    sorted_emb64 = emb[order].astype(np.float64)
    sums = np.add.reduceat(sorted_emb64, starts[:-1], axis=0)
    sums[counts == 0] = 0.0

    host_gram = np.zeros((S, D, D), np.float64)
    # per-(core, subject) row counts: split n_s into 8 near-equal parts
    part = np.zeros((NCORES, S), np.int64)
    if cap_tiles is not None:
        cap = cap_tiles * 128
        for s in range(S):
            dev_n = min(int(counts[s]), NCORES * cap)
            q, r = divmod(dev_n, NCORES)
            part[:, s] = q
            part[:r, s] += 1
            if counts[s] > dev_n:
                surplus = order[starts[s] + dev_n : starts[s + 1]]
                xs = emb[surplus].astype(np.float64)
                host_gram[s] = xs.T @ xs
        tiles_per_subject = [cap_tiles] * S
    else:
        for s in range(S):
            q, r = divmod(int(counts[s]), NCORES)
            part[:, s] = q
            part[:r, s] += 1
        # tiles per subject: identical across cores, padded to an EVEN count
        # so tile-pairs never mix subjects or straddle chunk boundaries
        tiles_per_subject = []
        for s in range(S):
            t = max(2, -(-int(part[:, s].max()) // 128))
            t += t % 2
            tiles_per_subject.append(t)
    T = sum(tiles_per_subject)
    if ctile is None:
        ctile, nchunks = _choose_ctile(T)
    else:
        nchunks = -(-T // ctile)
    # pad the total tile count to a chunk multiple: extra all-zero tiles are
    # appended to subject 15's accumulation group (they contribute zero)
    tiles_per_subject[S - 1] += nchunks * ctile - T
    T = nchunks * ctile

    emb_q = emb.astype(dt_np)

    tile_base = np.concatenate([[0], np.cumsum(tiles_per_subject)])
    shards = []
    for k in range(NCORES):
        arr = np.zeros((T * 128, D), dtype=dt_np)
        for s in range(S):
            off = int(starts[s] + part[:k, s].sum())
            n_ks = int(part[k, s])
            rows = order[off : off + n_ks]
            pos = int(tile_base[s]) * 128
            arr[pos : pos + n_ks] = emb_q[rows]
        if mode in ("fwr", "mm_fwr", "fwr8", "mm_fwr8", "drr", "mm_drr"):
            # permute pair blocks into the round-robin walk order so the
            # device schedule's pair p sits at layout position p
            _, perm, _ = _rr_pairs(tiles_per_subject,
                                   wide=mode in ("fwr8", "mm_fwr8"))
            arr = np.ascontiguousarray(
                arr.reshape(T // 2, 256, D)[np.asarray(perm)]
            ).reshape(T * 128, D)
        # chunk-partition-major layout: [nchunks, 128, ctile, D] where
        # dram[ch, p, c, e] = row (ch*ctile + c)*128 + p
        arr = np.ascontiguousarray(
            arr.reshape(nchunks, ctile, 128, D).transpose(0, 2, 1, 3)
        )
        shards.append(arr)
    return shards, counts, sums, host_gram, tiles_per_subject, nchunks


def _finalize(partials, counts, sums, host_gram=None):
    """Reduce per-core gram partials and run the tiny [S,S] pairwise stage."""
    p0 = np.asarray(partials[0])
    outp = p0.reshape(-1, S, D).shape[0]
    tot = np.zeros((outp, S, D), np.float64)
    for p in partials:
        tot += np.asarray(p, np.float64).reshape(outp, S, D)
    if outp == 2 * D:  # column-tiled: pair halves in partition halves
        tot = tot[:D] + tot[D:]
    G = tot.transpose(1, 0, 2)  # [S, 64, 64]
    if host_gram is not None:
        G = G + host_gram
    n = counts.astype(np.float64)

    means = sums / np.maximum(n, 1.0)[:, None]
    denom = np.maximum(n - 1.0, 1.0)[:, None, None]
    cov = (G - n[:, None, None] * means[:, :, None] * means[:, None, :]) / denom
    # (+ LAM * I cancels in the pairwise differences, as in the reference)
    iu, ju = np.triu_indices(S, k=1)
    diff = cov[iu] - cov[ju]
    fro2 = np.sum(diff * diff, axis=(1, 2))
    valid = n >= 2.0
    pv = valid[iu] & valid[ju]
    vals = np.sqrt(np.where(pv, fro2, 1.0))
    total = np.sum(np.where(pv, vals, 0.0))
    cnt = int(pv.sum())
    loss = total / max(cnt, 1) if cnt > 0 else 0.0
    return np.float32(loss)


def kernel(embeddings, subject_ids):
    emb = np.asarray(embeddings)
    shards, counts, sums, host_gram, tiles_per_subject, nchunks = (
        _prepare_shards(emb, subject_ids, ctile=CTILE)
    )
    nc = _build_nc(tiles_per_subject, nchunks, nq=NQ)
    in_maps = [{"x": shards[k]} for k in range(NCORES)]
    res = run_bass_kernel_spmd(nc, in_maps, list(range(NCORES)))
    partials = [res.results[k][name] for name in ("out", "out2")
                for k in range(NCORES) if name in res.results[k]]
    return _finalize(partials, counts, sums, host_gram)

